# revision 1
# baseline (speedup 1.0000x reference)
"""Trainium2 Bass kernel for MiniSelectiveSSM.

Reference computation (per batch row b):
    a = sigmoid(x @ Wa + ba)          # (T, N)
    u = (1 - a) * (x @ Wb + bb)       # (T, N)
    c = tanh(x @ Wc + bc)             # (T, N)
    s_t = a_t * s_{t-1} + u_t         # scan over T
    y = (c * s) @ Wy + by + x @ Wd + bd   # (T, D)

Sharding: data-parallel over batch B=8 across the 8 NeuronCores (one batch
row per core); projection weights replicated; the time scan stays local.

Layout: everything on-device is "transposed" — channels on partitions, time
on the free dimension. The host feeds x[b].T (D, T) so every GEMM contracts
over the partition dim with no on-device transposes, and the T-recurrence
maps directly onto the DVE's native tensor_tensor_scan instruction
(state = data0*state + data1 along the free dim, one recurrence per
partition).
"""

import os
import sys

import numpy as np


def _ensure_paths():
    for p in ("/opt/trn_rl_repo", "/root/.axon_site/_ro/trn_rl_repo"):
        if os.path.isdir(p) and p not in sys.path:
            sys.path.insert(0, p)


_ensure_paths()

import concourse.bass as bass  # noqa: E402
import concourse.tile as tile  # noqa: E402
from concourse import bacc, mybir  # noqa: E402
from concourse.bass_utils import run_bass_kernel_spmd  # noqa: E402

# Problem shapes (hardcoded per contract).
B, T, D, N = 8, 2048, 1024, 256
NCORES = 8
P = 128
KD = D // P   # 8  K-tiles over D
KN = N // P   # 2  K-tiles over N
TB = 512      # T-block (matmul moving free dim)
NB = T // TB  # 4 blocks

F32 = mybir.dt.float32
ALU = mybir.AluOpType
AF = mybir.ActivationFunctionType

# Matmul operand dtype: "f32" (exact, 4 cyc/row) or "f32r" (replicated fp32,
# 1 cyc/row at moving>=256, near-fp32 precision).
MM_DT = os.environ.get("SSM_MM_DT", "f32r")


def _mm(ap):
    if MM_DT == "f32r":
        return ap.bitcast(mybir.dt.float32r)
    return ap


def build_nc():
    nc = bacc.Bacc("TRN2", target_bir_lowering=False, debug=False)

    xT = nc.dram_tensor("xT", [D, T], F32, kind="ExternalInput")
    Wa = nc.dram_tensor("Wa", [D, N], F32, kind="ExternalInput")
    Wb = nc.dram_tensor("Wb", [D, N], F32, kind="ExternalInput")
    Wc = nc.dram_tensor("Wc", [D, N], F32, kind="ExternalInput")
    Wd = nc.dram_tensor("Wd", [D, D], F32, kind="ExternalInput")
    Wy = nc.dram_tensor("Wy", [N, D], F32, kind="ExternalInput")
    # Biases pre-shaped host-side to [P, groups]: col h holds bias[h*128+p].
    ba2 = nc.dram_tensor("ba2", [P, KN], F32, kind="ExternalInput")
    nba2 = nc.dram_tensor("nba2", [P, KN], F32, kind="ExternalInput")
    bb2 = nc.dram_tensor("bb2", [P, KN], F32, kind="ExternalInput")
    bc2 = nc.dram_tensor("bc2", [P, KN], F32, kind="ExternalInput")
    bY8 = nc.dram_tensor("bY8", [P, KD], F32, kind="ExternalInput")  # by + bd

    yT = nc.dram_tensor("yT", [D, T], F32, kind="ExternalOutput")

    xT_r = xT.ap().rearrange("(k p) t -> p k t", p=P)
    yT_r = yT.ap().rearrange("(m p) t -> p m t", p=P)

    with tile.TileContext(nc) as tc:
        with (
            tc.tile_pool(name="wpool", bufs=1) as wpool,
            tc.tile_pool(name="xpool", bufs=2) as xpool,
            tc.tile_pool(name="gpool", bufs=2) as gpool,
            tc.tile_pool(name="ypool", bufs=2) as ypool,
            tc.tile_pool(name="ps_abc", bufs=3, space="PSUM") as ps_abc,
            tc.tile_pool(name="ps_y", bufs=4, space="PSUM") as ps_y,
        ):
            # ---- replicated weights / biases into SBUF (once) ----
            wa_sb = wpool.tile([P, KD, N], F32)
            nc.sync.dma_start(wa_sb[:], Wa.ap().rearrange("(k p) n -> p k n", p=P))
            wb_sb = wpool.tile([P, KD, N], F32)
            nc.sync.dma_start(wb_sb[:], Wb.ap().rearrange("(k p) n -> p k n", p=P))
            wc_sb = wpool.tile([P, KD, N], F32)
            nc.sync.dma_start(wc_sb[:], Wc.ap().rearrange("(k p) n -> p k n", p=P))
            wd_sb = wpool.tile([P, KD, D], F32)
            nc.sync.dma_start(wd_sb[:], Wd.ap().rearrange("(k p) n -> p k n", p=P))
            wy_sb = wpool.tile([P, KN, D], F32)
            nc.sync.dma_start(wy_sb[:], Wy.ap().rearrange("(k p) n -> p k n", p=P))

            ba_sb = wpool.tile([P, KN], F32)
            nc.sync.dma_start(ba_sb[:], ba2.ap())
            nba_sb = wpool.tile([P, KN], F32)
            nc.sync.dma_start(nba_sb[:], nba2.ap())
            bb_sb = wpool.tile([P, KN], F32)
            nc.sync.dma_start(bb_sb[:], bb2.ap())
            bc_sb = wpool.tile([P, KN], F32)
            nc.sync.dma_start(bc_sb[:], bc2.ap())
            by_sb = wpool.tile([P, KD], F32)
            nc.sync.dma_start(by_sb[:], bY8.ap())

            s_prev = None  # carried scan state: previous block's s tile

            for blk in range(NB):
                tcol = slice(blk * TB, (blk + 1) * TB)

                x_sb = xpool.tile([P, KD, TB], F32, name=f"x_sb_{blk}", tag="x_sb")
                nc.sync.dma_start(x_sb[:], xT_r[:, :, tcol])

                a_t = gpool.tile([P, KN, TB], F32, name=f"a_{blk}", tag="a")
                am1_t = gpool.tile([P, KN, TB], F32, name=f"am1_{blk}", tag="am1")
                u_t = gpool.tile([P, KN, TB], F32, name=f"u_{blk}", tag="u")
                c_t = gpool.tile([P, KN, TB], F32, name=f"c_{blk}", tag="c")
                s_t = gpool.tile([P, KN, TB], F32, name=f"s_{blk}", tag="s")
                cs_t = gpool.tile([P, KN, TB], F32, name=f"cs_{blk}", tag="cs")

                # ---- gate GEMMs: zA/zB/zC = W.T @ xT-block ----
                for wsb, kind in ((wa_sb, "a"), (wb_sb, "b"), (wc_sb, "c")):
                    for m in range(KN):
                        mcol = slice(m * P, (m + 1) * P)
                        ps = ps_abc.tile(
                            [P, TB], F32, name=f"ps_{kind}{m}_{blk}", tag="ps_abc"
                        )
                        for k in range(KD):
                            nc.tensor.matmul(
                                ps[:],
                                _mm(wsb[:, k, mcol]),
                                _mm(x_sb[:, k, :]),
                                start=(k == 0),
                                stop=(k == KD - 1),
                            )
                        if kind == "a":
                            nc.scalar.activation(
                                a_t[:, m, :], ps[:], AF.Sigmoid,
                                bias=ba_sb[:, m : m + 1], scale=1.0,
                            )
                            nc.scalar.activation(
                                am1_t[:, m, :], ps[:], AF.Sigmoid,
                                bias=nba_sb[:, m : m + 1], scale=-1.0,
                            )
                        elif kind == "b":
                            # u = (zB + bb) * (1 - a), straight from PSUM on DVE
                            nc.vector.scalar_tensor_tensor(
                                u_t[:, m, :], ps[:], bb_sb[:, m : m + 1],
                                am1_t[:, m, :], op0=ALU.add, op1=ALU.mult,
                            )
                        else:
                            nc.scalar.activation(
                                c_t[:, m, :], ps[:], AF.Tanh,
                                bias=bc_sb[:, m : m + 1], scale=1.0,
                            )

                # ---- the time recurrence: one native scan per N-half ----
                for m in range(KN):
                    init = 0.0 if s_prev is None else s_prev[:, m, TB - 1 : TB]
                    nc.vector.tensor_tensor_scan(
                        s_t[:, m, :], a_t[:, m, :], u_t[:, m, :], init,
                        op0=ALU.mult, op1=ALU.add,
                    )
                nc.vector.tensor_tensor(cs_t[:], c_t[:], s_t[:], ALU.mult)
                s_prev = s_t

                # ---- output GEMM: yT = Wd.T@xT + Wy.T@cs (+ by + bd) ----
                y_sb = ypool.tile([P, KD, TB], F32, name=f"y_sb_{blk}", tag="y_sb")
                for m in range(KD):
                    mcol = slice(m * P, (m + 1) * P)
                    ps = ps_y.tile([P, TB], F32, name=f"ps_y{m}_{blk}", tag="ps_y")
                    for k in range(KD):
                        nc.tensor.matmul(
                            ps[:],
                            _mm(wd_sb[:, k, mcol]),
                            _mm(x_sb[:, k, :]),
                            start=(k == 0),
                            stop=False,
                        )
                    for k in range(KN):
                        nc.tensor.matmul(
                            ps[:],
                            _mm(wy_sb[:, k, mcol]),
                            _mm(cs_t[:, k, :]),
                            start=False,
                            stop=(k == KN - 1),
                        )
                    nc.scalar.activation(
                        y_sb[:, m, :], ps[:], AF.Identity,
                        bias=by_sb[:, m : m + 1], scale=1.0,
                    )
                nc.sync.dma_start(yT_r[:, :, tcol], y_sb[:])

    nc.compile()
    return nc


_NC_CACHE = {}


def _get_nc():
    key = MM_DT
    if key not in _NC_CACHE:
        _NC_CACHE[key] = build_nc()
    return _NC_CACHE[key]


def make_in_maps(x, Wa, ba, Wb, bb, Wc, bc, Wd, bd, Wy, by):
    x = np.asarray(x, np.float32)
    f = np.float32
    ba2 = np.ascontiguousarray(np.asarray(ba, f).reshape(KN, P).T)
    nba2 = np.ascontiguousarray(-np.asarray(ba, f).reshape(KN, P).T)
    bb2 = np.ascontiguousarray(np.asarray(bb, f).reshape(KN, P).T)
    bc2 = np.ascontiguousarray(np.asarray(bc, f).reshape(KN, P).T)
    bY8 = np.ascontiguousarray(
        (np.asarray(by, f) + np.asarray(bd, f)).reshape(KD, P).T
    )
    shared = {
        "Wa": np.ascontiguousarray(np.asarray(Wa, f)),
        "Wb": np.ascontiguousarray(np.asarray(Wb, f)),
        "Wc": np.ascontiguousarray(np.asarray(Wc, f)),
        "Wd": np.ascontiguousarray(np.asarray(Wd, f)),
        "Wy": np.ascontiguousarray(np.asarray(Wy, f)),
        "ba2": ba2, "nba2": nba2, "bb2": bb2, "bc2": bc2, "bY8": bY8,
    }
    return [
        {"xT": np.ascontiguousarray(x[b].T), **shared} for b in range(NCORES)
    ]


def kernel(x, Wa, ba, Wb, bb, Wc, bc, Wd, bd, Wy, by):
    nc = _get_nc()
    in_maps = make_in_maps(x, Wa, ba, Wb, bb, Wc, bc, Wd, bd, Wy, by)
    res = run_bass_kernel_spmd(nc, in_maps, core_ids=list(range(NCORES)))
    y = np.stack([res.results[b]["yT"].T for b in range(NCORES)], axis=0)
    return np.ascontiguousarray(y.astype(np.float32))


if __name__ == "__main__":
    rng = np.random.default_rng(0)
    sD = 1.0 / np.sqrt(D)
    sN = 1.0 / np.sqrt(N)
    inputs = {
        "x": rng.standard_normal((B, T, D), dtype=np.float32),
        "Wa": rng.standard_normal((D, N), dtype=np.float32) * sD,
        "ba": np.zeros(N, np.float32),
        "Wb": rng.standard_normal((D, N), dtype=np.float32) * sD,
        "bb": np.zeros(N, np.float32),
        "Wc": rng.standard_normal((D, N), dtype=np.float32) * sD,
        "bc": np.zeros(N, np.float32),
        "Wd": rng.standard_normal((D, D), dtype=np.float32) * sD,
        "bd": np.zeros(D, np.float32),
        "Wy": rng.standard_normal((N, D), dtype=np.float32) * sN,
        "by": np.zeros(D, np.float32),
    }
    y = kernel(**inputs)
    print("y", y.shape, y.dtype, float(np.abs(y).max()))
